# revision 10
# baseline (speedup 1.0000x reference)
"""MAB (multihead attention block) Trainium2 Bass kernel, v3.

Shards the B=4, N=2048 problem across 8 NeuronCores as (batch, query-half):
core c handles batch b = c//2, query rows [(c%2)*1024, (c%2)*1024+1024).

Reference quirk (faithful to the torch module): attention head h is masked
with adj_mask[h] (repeat_interleave on a head-major batch with B == H == 4),
so every core needs the n-slice of ALL FOUR adj_mask heads.

v3 architecture (all chosen to keep the PE array streaming at high duty so
the HAM clock gate stays at 2.4 GHz, and the ACT engine -- the exp
bottleneck at ~74us -- is never starved):
  - Scores run mc-major per head: one K-chunk weight load serves N=512
    streaming matmuls over all 8 query tiles (128 score MMs total, heads
    live at partition 32h of KpT/QpT via 32-row PE tiles).
  - exp: ACT drains each [128,1024] PSUM score tile straight to the
    per-head P tile [128, mc, 1024] in bf16 (evacuation + exp fused);
    softmax denominator via a ones-column in the V operand.
  - Mask: bf16 {0,-1000} in HBM, DMA accum_op=add onto the exp'd P
    (the only CCE op walrus accepts), then in-place relu on DVE (4x mode).
  - PV is V-stationary: 33-column weight loads, N=512 streaming matmuls
    accumulating O^T [33,512] in PSUM; transposed back per query tile with
    a tiny PE transpose; epilogue fused to one scalar_tensor_tensor:
    O = (P@Vaug)*rho + (Qp + bq + bv)   (bv folded into the residual).
  - Tail: LN -> FFN with g0 folded into W1 (host), be0@W1+br1 folded into
    the hidden bias, be0+br2 folded into one residual vector; FFN runs
    transposed (one PE transpose); rstd = exp(-0.5*ln(var+eps)) so every
    ACT function lives in the natural_log_exp_and_others table set (a
    single ACT_TABLE_LOAD for the whole kernel); output stored bf16.
"""

import numpy as np
import ml_dtypes

import concourse.bass as bass
import concourse.tile as tile
from concourse import bacc
from concourse import mybir
from concourse.bass import ds, ts
from concourse.bass_utils import run_bass_kernel_spmd
from concourse.masks import make_identity

BF16 = mybir.dt.bfloat16
F32 = mybir.dt.float32

B, N, M, D = 4, 2048, 2048, 128
H, DH = 4, 32
NLOC = N // 2          # query rows per core
QT = NLOC // 128       # query tiles per core (8)
MC = M // 128          # m chunks (16)
SCALE = 1.0 / np.sqrt(np.float32(DH))
N_CORES = 8


def _build_bass():
    # Force the activation-table chooser onto the one set that covers every
    # ACT function this kernel uses (exp, ln, identity, relu, copy): blank
    # the competing sets so Exp and Ln never thrash between two tables.
    # Names/order are preserved so act_func_set_id indices stay valid.
    if not getattr(bacc, "_mab_tables_patched", False):
        _orig_get_tables = bacc.get_activation_tables

        def _patched_get_tables(module_arch):
            tabs = _orig_get_tables(module_arch)
            keep = "natural_log_exp_and_others"
            if keep in tabs:
                need = {mybir.ActivationFunctionType.Exp,
                        mybir.ActivationFunctionType.Ln}
                if need <= tabs[keep]:
                    tabs = {name: (fns if name == keep else set())
                            for name, fns in tabs.items()}
            return tabs

        bacc.get_activation_tables = _patched_get_tables
        bacc._mab_tables_patched = True
    nc = bacc.Bacc("TRN2", target_bir_lowering=False, debug=False,
                   num_devices=N_CORES)

    # ---- I/O ----
    KT_d = nc.dram_tensor("KT", [D, M], F32, kind="ExternalInput").ap()
    QT_d = nc.dram_tensor("QTr", [D, NLOC], F32, kind="ExternalInput").ap()
    # mask bias {0,-1000}: [h, mc, p, qt*128+j]
    MSK_d = nc.dram_tensor("maskT", [H, MC, 128, NLOC], BF16,
                           kind="ExternalInput").ap()
    Wq_d = nc.dram_tensor("Wq", [D, D], F32, kind="ExternalInput").ap()
    Wk_d = nc.dram_tensor("Wk", [D, D], F32, kind="ExternalInput").ap()
    Wv_d = nc.dram_tensor("Wv", [D, D], F32, kind="ExternalInput").ap()
    W1p_d = nc.dram_tensor("W1p", [D, D], BF16, kind="ExternalInput").ap()
    Wr2_d = nc.dram_tensor("Wr2b", [D, D], BF16, kind="ExternalInput").ap()
    cols_d = {}
    for nm in ["bk", "bq_s", "b1p"]:
        cols_d[nm] = nc.dram_tensor(nm, [D, 1], F32, kind="ExternalInput").ap()
    BQV_d = nc.dram_tensor("bqv", [1, D], F32, kind="ExternalInput").ap()
    vecs_d = {}
    for nm in ["g0", "bb", "g1", "be1"]:
        vecs_d[nm] = nc.dram_tensor(nm, [1, D], BF16, kind="ExternalInput").ap()
    out_d = nc.dram_tensor("out", [NLOC, D], BF16, kind="ExternalOutput").ap()

    with tile.TileContext(nc) as tc:
        _emit(tc, KT_d, QT_d, MSK_d, Wq_d, Wk_d, Wv_d, W1p_d, Wr2_d,
              cols_d, BQV_d, vecs_d, out_d)
    nc.compile()
    return nc


def _emit(tc, KT_d, QT_d, MSK_d, Wq_d, Wk_d, Wv_d, W1p_d, Wr2_d,
          cols_d, BQV_d, vecs_d, out_d):
    nc = tc.nc
    from contextlib import ExitStack
    ctx = ExitStack()
    singles = ctx.enter_context(tc.tile_pool(name="singles", bufs=1))
    ppool = ctx.enter_context(tc.tile_pool(name="ppool", bufs=2))
    otpool = ctx.enter_context(tc.tile_pool(name="otpool", bufs=2))
    tpool = ctx.enter_context(tc.tile_pool(name="tailsb", bufs=2))
    small = ctx.enter_context(tc.tile_pool(name="small", bufs=4))
    # PSUM: scores 2x2 banks + po 2x1 + shared transpose/tail/proj 2x1 = 8
    scp = ctx.enter_context(tc.tile_pool(name="scp", bufs=2, space="PSUM"))
    pvp = ctx.enter_context(tc.tile_pool(name="pvp", bufs=2, space="PSUM"))
    ttp = ctx.enter_context(tc.tile_pool(name="ttp", bufs=2, space="PSUM"))

    # ---- persistent SBUF ----
    KT = singles.tile([D, M], F32)          # K[b]^T
    QTt = singles.tile([D, NLOC], F32)      # Q-slice^T
    Wq = singles.tile([D, D], F32)
    Wk = singles.tile([D, D], F32)
    Wv = singles.tile([D, D], F32)
    W1p = singles.tile([D, D], BF16)        # g0-folded Wr1
    Wr2 = singles.tile([D, D], BF16)
    cols = {nm: singles.tile([D, 1], F32, tag=f"col_{nm}", name=f"col_{nm}")
            for nm in cols_d}
    BQV = singles.tile([128, D], F32)       # bq + bv broadcast
    vecs = {nm: singles.tile([128, D], BF16, tag=f"vec_{nm}", name=f"vec_{nm}")
            for nm in vecs_d}
    KpT = singles.tile([D, M], BF16)        # (K@Wk+bk)^T, head h at part 32h
    QpT = singles.tile([D, NLOC], BF16)     # scaled (Q@Wq+bq)^T
    Vaug = singles.tile([128, MC, H, 34], BF16)  # [.,mc,h,0:32]=V, 32=ones
    Qn = singles.tile([128, QT, D], F32)    # Q@Wq + bq + bv (residual)
    Ofull = singles.tile([128, QT, D], F32)
    ident_b = singles.tile([128, 128], BF16)
    eps_t = singles.tile([128, 1], F32)

    make_identity(nc, ident_b)
    nc.vector.memset(eps_t, 1e-5)
    nc.gpsimd.memset(Vaug, 0.0)
    nc.vector.memset(Vaug[:, :, :, 32:33], 1.0)

    # ---- const loads (HWDGE for bulk, SWDGE for broadcasts) ----
    nc.sync.dma_start(KT, KT_d)
    nc.sync.dma_start(QTt, QT_d)
    nc.sync.dma_start(Wq, Wq_d)
    nc.sync.dma_start(Wk, Wk_d)
    nc.sync.dma_start(Wv, Wv_d)
    nc.sync.dma_start(W1p, W1p_d)
    nc.sync.dma_start(Wr2, Wr2_d)
    for nm in cols_d:
        nc.sync.dma_start(cols[nm], cols_d[nm])
    bq_ap = bass.AP(tensor=BQV_d.tensor, offset=BQV_d.offset,
                    ap=[[0, 128], BQV_d.ap[1]])
    nc.gpsimd.dma_start(out=BQV, in_=bq_ap)
    for nm in vecs_d:
        vap = bass.AP(tensor=vecs_d[nm].tensor, offset=vecs_d[nm].offset,
                      ap=[[0, 128], vecs_d[nm].ap[1]])
        nc.gpsimd.dma_start(out=vecs[nm], in_=vap)

    # ---- projections ----
    # KpT[dv, m] = Wk^T @ KT + bk
    for j in range(4):
        ps = ttp.tile([128, 512], F32, tag="tt", name=f"pk{j}")
        nc.tensor.matmul(ps, Wk, KT[:, ts(j, 512)], start=True, stop=True)
        nc.vector.tensor_scalar(KpT[:, ts(j, 512)], ps, cols["bk"], None,
                                mybir.AluOpType.add)
    # QpT scaled by 1/sqrt(dh); bias pre-scaled on host (bq_s)
    for j in range(2):
        ps = ttp.tile([128, 512], F32, tag="tt", name=f"pq{j}")
        nc.tensor.matmul(ps, Wq, QTt[:, ts(j, 512)], start=True, stop=True)
        nc.vector.tensor_scalar(QpT[:, ts(j, 512)], ps, float(SCALE),
                                cols["bq_s"], mybir.AluOpType.mult,
                                mybir.AluOpType.add)
    # V natural (no bias -- bv is folded into the residual Qn)
    for mc in range(MC):
        ps = ttp.tile([128, 512], F32, tag="tt", name=f"pv{mc}")
        nc.tensor.matmul(ps[:, :128], KT[:, ts(mc, 128)], Wv,
                         start=True, stop=True)
        nc.vector.tensor_copy(out=Vaug[:, mc, :, 0:32],
                              in_=ps[:, 0:128].rearrange("p (h d) -> p h d",
                                                         h=4))
    # Qn = Q@Wq + (bq+bv)  (residual path, fp32)
    for qt in range(QT):
        ps = ttp.tile([128, 512], F32, tag="tt", name=f"pn{qt}")
        nc.tensor.matmul(ps[:, :128], QTt[:, ts(qt, 128)], Wq,
                         start=True, stop=True)
        nc.vector.tensor_tensor(Qn[:, qt, :], ps[:, :128], BQV,
                                mybir.AluOpType.add)

    # ---- attention: heads in pairs, mc-major scores, V-stationary PV ----
    def pv_and_epilogue(h, P_h):
        for nb in range(2):
            po = pvp.tile([128, 512], F32, tag="po", name=f"po{h}_{nb}")
            for mc in range(MC):
                nc.tensor.matmul(po[0:33, :], Vaug[:, mc, h, 0:33],
                                 P_h[:, mc, ts(nb, 512)],
                                 start=(mc == 0), stop=(mc == MC - 1))
            # O^T [33, 512] -> SBUF, then per-qt transpose + epilogue
            OT = otpool.tile([128, 512], BF16, tag="ot", name=f"ot{h}_{nb}")
            nc.vector.tensor_copy(out=OT[0:33, :], in_=po[0:33, :])
            for ql in range(4):
                qt = nb * 4 + ql
                tr = ttp.tile([128, 512], F32, tag="tt", name=f"tr{h}_{qt}")
                trb = tr.bitcast(BF16)
                nc.tensor.transpose(trb[:, 0:33], OT[0:33, ts(ql, 128)],
                                    ident_b[0:33, 0:33])
                rho = small.tile([128, 1], F32, tag="rho", name=f"rho{h}{qt}")
                nc.vector.reciprocal(rho, trb[:, 32:33])
                nc.vector.scalar_tensor_tensor(
                    Ofull[:, qt, ts(h, DH)], trb[:, 0:32], rho,
                    Qn[:, qt, ts(h, DH)], mybir.AluOpType.mult,
                    mybir.AluOpType.add)

    for pair in range(2):
        h0, h1 = 2 * pair, 2 * pair + 1
        Ps = {h: ppool.tile([128, MC, NLOC], BF16, tag=f"p{h % 2}",
                            name=f"p{h}") for h in (h0, h1)}
        for mc in range(MC):
            for h in (h0, h1):
                t = scp.tile([128, 2, 512], F32, tag="sc",
                             name=f"sc{h}_{mc}")
                for k in range(2):
                    nc.tensor.matmul(t[:, k, :],
                                     KpT[ts(h, DH), ts(mc, 128)],
                                     QpT[ts(h, DH), ts(k, 512)],
                                     start=True, stop=True,
                                     tile_position=(32 * h, 0))
                nc.scalar.activation(Ps[h][:, mc, :],
                                     t.rearrange("p a b -> p (a b)"),
                                     mybir.ActivationFunctionType.Exp)
                nc.gpsimd.dma_start(out=Ps[h][:, mc, :], in_=MSK_d[h, mc],
                                    accum_op=mybir.AluOpType.add)
                nc.vector.tensor_scalar_max(Ps[h][:, mc, :],
                                            Ps[h][:, mc, :], 0.0)
        pv_and_epilogue(h0, Ps[h0])
        pv_and_epilogue(h1, Ps[h1])

    # ---- tail: LN0 -> FFN -> LN1 -> out ----
    for qt in range(QT):
        x = Ofull[:, qt, :]
        st = small.tile([128, 6], F32, tag="bnst", name=f"st{qt}")
        mv = small.tile([128, 2], F32, tag="bnmv", name=f"mv{qt}")
        nc.vector.bn_stats(st, x)
        nc.vector.bn_aggr(mv, st)
        # rstd = exp(-0.5*ln(var+eps)): stays in the exp/ln table set
        sd = small.tile([128, 1], F32, tag="sd", name=f"sd{qt}")
        nc.scalar.activation(sd, mv[:, 1:2],
                             mybir.ActivationFunctionType.Ln, bias=eps_t)
        rstd = small.tile([128, 1], F32, tag="rstd", name=f"rs{qt}")
        nc.scalar.activation(rstd, sd,
                             mybir.ActivationFunctionType.Exp, scale=-0.5)
        z = tpool.tile([128, D], BF16, tag="z", name=f"z{qt}")
        nc.vector.tensor_scalar(z, x, mv[:, 0:1], rstd,
                                mybir.AluOpType.subtract,
                                mybir.AluOpType.mult)
        # FFN (transposed): hT = relu(W1p.T @ z.T + b1p); y = hT.T@Wr2 + r
        tp1 = ttp.tile([128, 512], F32, tag="tt", name=f"tt{qt}")
        tp1b = tp1.bitcast(BF16)
        nc.tensor.transpose(tp1b[:, 0:128], z, ident_b)
        zT = tpool.tile([128, D], BF16, tag="zT", name=f"zT{qt}")
        nc.scalar.copy(zT, tp1b[:, 0:128])
        nc.tensor.matmul(tp1[:, 128:256], W1p, zT, start=True, stop=True)
        h1T = tpool.tile([128, D], BF16, tag="h1T", name=f"h1T{qt}")
        nc.scalar.activation(h1T, tp1[:, 128:256],
                             mybir.ActivationFunctionType.Relu,
                             bias=cols["b1p"])
        nc.tensor.matmul(tp1[:, 256:384], h1T, Wr2, start=True, stop=True)
        r = tpool.tile([128, D], BF16, tag="r", name=f"r{qt}")
        nc.vector.tensor_tensor(r, z, vecs["g0"], mybir.AluOpType.mult)
        nc.vector.tensor_tensor(r, r, vecs["bb"], mybir.AluOpType.add)
        y = tpool.tile([128, D], BF16, tag="y", name=f"y{qt}")
        nc.vector.scalar_tensor_tensor(y, tp1[:, 256:384], 1.0, r,
                                       mybir.AluOpType.mult,
                                       mybir.AluOpType.add)
        # LN1
        st2 = small.tile([128, 6], F32, tag="bnst", name=f"st2_{qt}")
        mv2 = small.tile([128, 2], F32, tag="bnmv", name=f"mv2_{qt}")
        nc.vector.bn_stats(st2, y)
        nc.vector.bn_aggr(mv2, st2)
        sd2 = small.tile([128, 1], F32, tag="sd", name=f"sd2_{qt}")
        nc.scalar.activation(sd2, mv2[:, 1:2],
                             mybir.ActivationFunctionType.Ln, bias=eps_t)
        rstd2 = small.tile([128, 1], F32, tag="rstd", name=f"rs2_{qt}")
        nc.scalar.activation(rstd2, sd2,
                             mybir.ActivationFunctionType.Exp, scale=-0.5)
        z1 = tpool.tile([128, D], BF16, tag="z1", name=f"z1_{qt}")
        nc.vector.tensor_scalar(z1, y, mv2[:, 0:1], rstd2,
                                mybir.AluOpType.subtract,
                                mybir.AluOpType.mult)
        o = tpool.tile([128, D], BF16, tag="o", name=f"o{qt}")
        nc.vector.tensor_tensor(o, z1, vecs["g1"], mybir.AluOpType.mult)
        nc.vector.tensor_tensor(o, o, vecs["be1"], mybir.AluOpType.add)
        nc.sync.dma_start(out_d[ts(qt, 128), :], o)

    ctx.close()


_NC_CACHE = {}


def _get_nc():
    if "nc" not in _NC_CACHE:
        _NC_CACHE["nc"] = _build_bass()
    return _NC_CACHE["nc"]


def _prep_inputs(Q, K, adj_mask, Wq, bq, Wk, bk, Wv, bv, Wr1, br1, Wr2, br2,
                 g0, be0, g1, be1):
    bf = ml_dtypes.bfloat16
    f32 = np.float32
    Q = np.asarray(Q, f32)
    K = np.asarray(K, f32)
    adj = np.asarray(adj_mask)
    g0f = np.asarray(g0, f32)
    be0f = np.asarray(be0, f32)
    Wr1f = np.asarray(Wr1, f32)
    shared = {
        "Wq": np.ascontiguousarray(Wq, f32),
        "Wk": np.ascontiguousarray(Wk, f32),
        "Wv": np.ascontiguousarray(Wv, f32),
        # g0 folded into Wr1; be0@Wr1+br1 folded into hidden bias
        "W1p": np.ascontiguousarray(g0f[:, None] * Wr1f).astype(bf),
        "Wr2b": np.ascontiguousarray(Wr2).astype(bf),
        "bk": np.ascontiguousarray(bk, f32).reshape(D, 1),
        "bq_s": (np.asarray(bq, f32) * SCALE).reshape(D, 1).copy(),
        "b1p": (be0f @ Wr1f + np.asarray(br1, f32)).reshape(D, 1).copy(),
        "bqv": (np.asarray(bq, f32) + np.asarray(bv, f32)).reshape(1, D),
        "g0": np.ascontiguousarray(g0, f32).reshape(1, D).astype(bf),
        "bb": (be0f + np.asarray(br2, f32)).reshape(1, D).astype(bf),
        "g1": np.ascontiguousarray(g1, f32).reshape(1, D).astype(bf),
        "be1": np.ascontiguousarray(be1, f32).reshape(1, D).astype(bf),
    }
    # mask layout per half: [h, mc, p, qt*128+j] = bias(adj[h, n0+qt*128+j,
    # mc*128+p]), stored bf16 {0,-1000} for the DMA-add masking
    mhalf = []
    for half in range(2):
        a = adj[:, half * NLOC:(half + 1) * NLOC, :]
        a = a.reshape(H, QT, 128, MC, 128)          # [h, qt, j, mc, p]
        a = np.ascontiguousarray(a.transpose(0, 3, 4, 1, 2))  # [h,mc,p,qt,j]
        a = a.reshape(H, MC, 128, NLOC)
        mhalf.append(np.where(a > 0, 0.0, -1000.0).astype(bf))
    in_maps = []
    for c in range(N_CORES):
        b, half = c // 2, c % 2
        im = dict(shared)
        im["KT"] = np.ascontiguousarray(K[b].T)
        im["QTr"] = np.ascontiguousarray(Q[b, half * NLOC:(half + 1) * NLOC].T)
        im["maskT"] = mhalf[half]
        in_maps.append(im)
    return in_maps


def _ensure_ntff_hook():
    """The agent image's antenv lacks axon_hooks, so the boot-time NTFF hook
    install silently degrades. Fabricate the module and install the hook via
    the boot module's own ctypes factory so trace=True works."""
    import sys
    import types
    try:
        from antenv.axon_hooks import get_axon_ntff_profile_hook  # noqa: F401
        return  # real module exists
    except ImportError:
        pass
    if "antenv.axon_hooks" in sys.modules:
        return
    from trn_agent_boot.trn_boot import _ntff_profile_via_ctypes
    hook = _ntff_profile_via_ctypes("/opt/axon/libaxon_pjrt.so")
    mod = types.ModuleType("antenv.axon_hooks")
    mod._hook = hook
    mod.get_axon_ntff_profile_hook = lambda: mod._hook
    mod.set_axon_ntff_profile_hook = lambda h: setattr(mod, "_hook", h)
    sys.modules["antenv.axon_hooks"] = mod


def run(trace=False, **inputs):
    nc = _get_nc()
    in_maps = _prep_inputs(**inputs)
    if trace:
        try:
            _ensure_ntff_hook()
        except Exception as e:
            print(f"ntff hook install failed ({e}); running without trace")
            trace = False
    res = run_bass_kernel_spmd(nc, in_maps, core_ids=list(range(N_CORES)),
                               trace=trace)
    out = np.empty((B, N, D), np.float32)
    for c in range(N_CORES):
        b, half = c // 2, c % 2
        out[b, half * NLOC:(half + 1) * NLOC] = \
            np.asarray(res.results[c]["out"], dtype=np.float32)
    return out, res


def kernel(**inputs) -> np.ndarray:
    out, _ = run(trace=False, **inputs)
    return out


# revision 11
# speedup vs baseline: 1.1053x; 1.1053x over previous
"""MAB (multihead attention block) Trainium2 Bass kernel, v3.

Shards the B=4, N=2048 problem across 8 NeuronCores as (batch, query-half):
core c handles batch b = c//2, query rows [(c%2)*1024, (c%2)*1024+1024).

Reference quirk (faithful to the torch module): attention head h is masked
with adj_mask[h] (repeat_interleave on a head-major batch with B == H == 4),
so every core needs the n-slice of ALL FOUR adj_mask heads.

v3 architecture (all chosen to keep the PE array streaming at high duty so
the HAM clock gate stays at 2.4 GHz, and the ACT engine -- the exp
bottleneck at ~74us -- is never starved):
  - Scores run mc-major per head: one K-chunk weight load serves N=512
    streaming matmuls over all 8 query tiles (128 score MMs total, heads
    live at partition 32h of KpT/QpT via 32-row PE tiles).
  - exp: ACT drains each [128,1024] PSUM score tile straight to the
    per-head P tile [128, mc, 1024] in bf16 (evacuation + exp fused);
    softmax denominator via a ones-column in the V operand.
  - Mask: bf16 {0,-1000} in HBM, DMA accum_op=add onto the exp'd P
    (the only CCE op walrus accepts), then in-place relu on DVE (4x mode).
  - PV is V-stationary: 33-column weight loads, N=512 streaming matmuls
    accumulating O^T [33,512] in PSUM; transposed back per query tile with
    a tiny PE transpose; epilogue fused to one scalar_tensor_tensor:
    O = (P@Vaug)*rho + (Qp + bq + bv)   (bv folded into the residual).
  - Tail: LN -> FFN with g0 folded into W1 (host), be0@W1+br1 folded into
    the hidden bias, be0+br2 folded into one residual vector; FFN runs
    transposed (one PE transpose); rstd = exp(-0.5*ln(var+eps)) so every
    ACT function lives in the natural_log_exp_and_others table set (a
    single ACT_TABLE_LOAD for the whole kernel); output stored bf16.
"""

import numpy as np
import ml_dtypes

import concourse.bass as bass
import concourse.tile as tile
from concourse import bacc
from concourse import mybir
from concourse.bass import ds, ts
from concourse.bass_utils import run_bass_kernel_spmd
from concourse.masks import make_identity

BF16 = mybir.dt.bfloat16
F32 = mybir.dt.float32

B, N, M, D = 4, 2048, 2048, 128
H, DH = 4, 32
NLOC = N // 2          # query rows per core
QT = NLOC // 128       # query tiles per core (8)
MC = M // 128          # m chunks (16)
SCALE = 1.0 / np.sqrt(np.float32(DH))
N_CORES = 8


def _build_bass():
    # Force the activation-table chooser onto the one set that covers every
    # ACT function this kernel uses (exp, ln, identity, relu, copy): blank
    # the competing sets so Exp and Ln never thrash between two tables.
    # Names/order are preserved so act_func_set_id indices stay valid.
    if not getattr(bacc, "_mab_tables_patched", False):
        _orig_get_tables = bacc.get_activation_tables

        def _patched_get_tables(module_arch):
            tabs = _orig_get_tables(module_arch)
            keep = "natural_log_exp_and_others"
            if keep in tabs:
                need = {mybir.ActivationFunctionType.Exp,
                        mybir.ActivationFunctionType.Ln}
                if need <= tabs[keep]:
                    tabs = {name: (fns if name == keep else set())
                            for name, fns in tabs.items()}
            return tabs

        bacc.get_activation_tables = _patched_get_tables
        bacc._mab_tables_patched = True
    nc = bacc.Bacc("TRN2", target_bir_lowering=False, debug=False,
                   num_devices=N_CORES)

    # ---- I/O ----
    KT_d = nc.dram_tensor("KT", [D, M], F32, kind="ExternalInput").ap()
    QT_d = nc.dram_tensor("QTr", [D, NLOC], F32, kind="ExternalInput").ap()
    # mask bias {0,-1000}: [h, mc, p, qt*128+j]
    MSK_d = nc.dram_tensor("maskT", [H, MC, 128, NLOC], BF16,
                           kind="ExternalInput").ap()
    Wq_d = nc.dram_tensor("Wq", [D, D], F32, kind="ExternalInput").ap()
    Wk_d = nc.dram_tensor("Wk", [D, D], F32, kind="ExternalInput").ap()
    Wv_d = nc.dram_tensor("Wv", [D, D], F32, kind="ExternalInput").ap()
    W1p_d = nc.dram_tensor("W1p", [D, D], BF16, kind="ExternalInput").ap()
    Wr2_d = nc.dram_tensor("Wr2b", [D, D], BF16, kind="ExternalInput").ap()
    cols_d = {}
    for nm in ["bk", "bq_s", "b1p"]:
        cols_d[nm] = nc.dram_tensor(nm, [D, 1], F32, kind="ExternalInput").ap()
    BQV_d = nc.dram_tensor("bqv", [1, D], F32, kind="ExternalInput").ap()
    vecs_d = {}
    for nm in ["g0", "bb", "g1", "be1"]:
        vecs_d[nm] = nc.dram_tensor(nm, [1, D], BF16, kind="ExternalInput").ap()
    out_d = nc.dram_tensor("out", [NLOC, D], BF16, kind="ExternalOutput").ap()

    with tile.TileContext(nc) as tc:
        _emit(tc, KT_d, QT_d, MSK_d, Wq_d, Wk_d, Wv_d, W1p_d, Wr2_d,
              cols_d, BQV_d, vecs_d, out_d)
    nc.compile()
    return nc


def _emit(tc, KT_d, QT_d, MSK_d, Wq_d, Wk_d, Wv_d, W1p_d, Wr2_d,
          cols_d, BQV_d, vecs_d, out_d):
    nc = tc.nc
    from contextlib import ExitStack
    ctx = ExitStack()
    singles = ctx.enter_context(tc.tile_pool(name="singles", bufs=1))
    ppool = ctx.enter_context(tc.tile_pool(name="ppool", bufs=2))
    otpool = ctx.enter_context(tc.tile_pool(name="otpool", bufs=2))
    tpool = ctx.enter_context(tc.tile_pool(name="tailsb", bufs=2))
    small = ctx.enter_context(tc.tile_pool(name="small", bufs=4))
    # PSUM: scores 2x2 banks + po/transpose/tail 4x1 banks = 8
    scp = ctx.enter_context(tc.tile_pool(name="scp", bufs=2, space="PSUM"))
    pvp = ctx.enter_context(tc.tile_pool(name="pvp", bufs=4, space="PSUM"))

    # ---- persistent SBUF ----
    KT = singles.tile([D, M], F32)          # K[b]^T
    QTt = singles.tile([D, NLOC], F32)      # Q-slice^T
    Wq = singles.tile([D, D], F32)
    Wk = singles.tile([D, D], F32)
    Wv = singles.tile([D, D], F32)
    W1p = singles.tile([D, D], BF16)        # g0-folded Wr1
    Wr2 = singles.tile([D, D], BF16)
    cols = {nm: singles.tile([D, 1], F32, tag=f"col_{nm}", name=f"col_{nm}")
            for nm in cols_d}
    BQV = singles.tile([128, D], F32)       # bq + bv broadcast
    vecs = {nm: singles.tile([128, D], BF16, tag=f"vec_{nm}", name=f"vec_{nm}")
            for nm in vecs_d}
    KpT = singles.tile([D, M], BF16)        # (K@Wk+bk)^T, head h at part 32h
    QpT = singles.tile([D, NLOC], BF16)     # scaled (Q@Wq+bq)^T
    Vaug = singles.tile([128, MC, H, 34], BF16)  # [.,mc,h,0:32]=V, 32=ones
    Qn = singles.tile([128, QT, D], F32)    # Q@Wq + bq + bv (residual)
    Ofull = singles.tile([128, QT, D], F32)
    ident_b = singles.tile([128, 128], BF16)
    eps_t = singles.tile([128, 1], F32)

    make_identity(nc, ident_b)
    nc.vector.memset(eps_t, 1e-5)
    nc.gpsimd.memset(Vaug, 0.0)
    nc.vector.memset(Vaug[:, :, :, 32:33], 1.0)

    # ---- const loads (HWDGE for bulk, SWDGE for broadcasts) ----
    nc.sync.dma_start(KT, KT_d)
    nc.sync.dma_start(QTt, QT_d)
    nc.sync.dma_start(Wq, Wq_d)
    nc.sync.dma_start(Wk, Wk_d)
    nc.sync.dma_start(Wv, Wv_d)
    nc.sync.dma_start(W1p, W1p_d)
    nc.sync.dma_start(Wr2, Wr2_d)
    for nm in cols_d:
        nc.sync.dma_start(cols[nm], cols_d[nm])
    bq_ap = bass.AP(tensor=BQV_d.tensor, offset=BQV_d.offset,
                    ap=[[0, 128], BQV_d.ap[1]])
    nc.gpsimd.dma_start(out=BQV, in_=bq_ap)
    for nm in vecs_d:
        vap = bass.AP(tensor=vecs_d[nm].tensor, offset=vecs_d[nm].offset,
                      ap=[[0, 128], vecs_d[nm].ap[1]])
        nc.gpsimd.dma_start(out=vecs[nm], in_=vap)

    # ---- projections ----
    # KpT[dv, m] = Wk^T @ KT + bk
    for j in range(4):
        ps = pvp.tile([128, 512], F32, tag="po", name=f"pk{j}")
        nc.tensor.matmul(ps, Wk, KT[:, ts(j, 512)], start=True, stop=True)
        nc.vector.tensor_scalar(KpT[:, ts(j, 512)], ps, cols["bk"], None,
                                mybir.AluOpType.add)
    # QpT scaled by 1/sqrt(dh); bias pre-scaled on host (bq_s)
    for j in range(2):
        ps = pvp.tile([128, 512], F32, tag="po", name=f"pq{j}")
        nc.tensor.matmul(ps, Wq, QTt[:, ts(j, 512)], start=True, stop=True)
        nc.vector.tensor_scalar(QpT[:, ts(j, 512)], ps, float(SCALE),
                                cols["bq_s"], mybir.AluOpType.mult,
                                mybir.AluOpType.add)
    # V natural (no bias -- bv is folded into the residual Qn)
    for mc in range(MC):
        ps = pvp.tile([128, 512], F32, tag="po", name=f"pv{mc}")
        nc.tensor.matmul(ps[:, :128], KT[:, ts(mc, 128)], Wv,
                         start=True, stop=True)
        nc.vector.tensor_copy(out=Vaug[:, mc, :, 0:32],
                              in_=ps[:, 0:128].rearrange("p (h d) -> p h d",
                                                         h=4))
    # Qn = Q@Wq + (bq+bv)  (residual path, fp32)
    for qt in range(QT):
        ps = pvp.tile([128, 512], F32, tag="po", name=f"pn{qt}")
        nc.tensor.matmul(ps[:, :128], QTt[:, ts(qt, 128)], Wq,
                         start=True, stop=True)
        nc.vector.tensor_tensor(Qn[:, qt, :], ps[:, :128], BQV,
                                mybir.AluOpType.add)

    # ---- attention: heads in pairs, mc-major scores, V-stationary PV ----
    # PV chunk matmuls are interleaved into the mc loop (P chunk mc is
    # consumed ~2 iterations after its scores) so the PE never idles long
    # enough for the HAM clock gate to re-throttle it to 1.2 GHz.
    for pair in range(2):
        h0, h1 = 2 * pair, 2 * pair + 1
        Ps = {h: ppool.tile([128, MC, NLOC], BF16, tag=f"p{h % 2}",
                            name=f"p{h}") for h in (h0, h1)}
        pos = {(h, nb): pvp.tile([128, 512], F32, tag="po",
                                 name=f"po{h}_{nb}")
               for h in (h0, h1) for nb in range(2)}
        for mc in range(MC):
            for h in (h0, h1):
                t = scp.tile([128, 2, 512], F32, tag="sc",
                             name=f"sc{h}_{mc}")
                for k in range(2):
                    nc.tensor.matmul(t[:, k, :],
                                     KpT[ts(h, DH), ts(mc, 128)],
                                     QpT[ts(h, DH), ts(k, 512)],
                                     start=True, stop=True,
                                     tile_position=(32 * h, 0))
                nc.scalar.activation(Ps[h][:, mc, :],
                                     t.rearrange("p a b -> p (a b)"),
                                     mybir.ActivationFunctionType.Exp)
                nc.gpsimd.dma_start(out=Ps[h][:, mc, :], in_=MSK_d[h, mc],
                                    accum_op=mybir.AluOpType.add)
                nc.vector.tensor_scalar_max(Ps[h][:, mc, :],
                                            Ps[h][:, mc, :], 0.0)
            for h in (h0, h1):
                for nb in range(2):
                    nc.tensor.matmul(pos[(h, nb)][0:33, :],
                                     Vaug[:, mc, h, 0:33],
                                     Ps[h][:, mc, ts(nb, 512)],
                                     start=(mc == 0), stop=(mc == MC - 1))
        # epilogues: evacuate O^T, transpose per qt, fused normalize+residual
        OTs = {}
        for h in (h0, h1):
            for nb in range(2):
                OT = otpool.tile([128, 512], BF16, tag=f"ot{nb}",
                                 name=f"ot{h}_{nb}")
                nc.vector.tensor_copy(out=OT[0:33, :],
                                      in_=pos[(h, nb)][0:33, :])
                OTs[(h, nb)] = OT
        for h in (h0, h1):
            for nb in range(2):
                for ql in range(4):
                    qt = nb * 4 + ql
                    tr = pvp.tile([128, 512], F32, tag="po",
                                  name=f"tr{h}_{qt}")
                    trb = tr.bitcast(BF16)
                    nc.tensor.transpose(trb[:, 0:33],
                                        OTs[(h, nb)][0:33, ts(ql, 128)],
                                        ident_b[0:33, 0:33])
                    rho = small.tile([128, 1], F32, tag="rho",
                                     name=f"rho{h}{qt}")
                    nc.vector.reciprocal(rho, trb[:, 32:33])
                    nc.vector.scalar_tensor_tensor(
                        Ofull[:, qt, ts(h, DH)], trb[:, 0:32], rho,
                        Qn[:, qt, ts(h, DH)], mybir.AluOpType.mult,
                        mybir.AluOpType.add)

    # ---- tail: LN0 -> FFN -> LN1 -> out (rstd ACT calls batched) ----
    mvall = singles.tile([128, QT, 2], F32)
    rstdall = singles.tile([128, QT], F32)
    mv2all = singles.tile([128, QT, 2], F32)
    rstd2all = singles.tile([128, QT], F32)
    Yfull = singles.tile([128, QT, D], BF16)
    for qt in range(QT):
        st = small.tile([128, 6], F32, tag="bnst", name=f"st{qt}")
        nc.vector.bn_stats(st, Ofull[:, qt, :])
        nc.vector.bn_aggr(mvall[:, qt, :], st)
    # rstd = exp(-0.5*ln(var+eps)): stays in the exp/ln table set
    nc.scalar.activation(rstdall, mvall[:, :, 1],
                         mybir.ActivationFunctionType.Ln, bias=eps_t)
    nc.scalar.activation(rstdall, rstdall,
                         mybir.ActivationFunctionType.Exp, scale=-0.5)
    for qt in range(QT):
        z = tpool.tile([128, D], BF16, tag="z", name=f"z{qt}")
        nc.vector.tensor_scalar(z, Ofull[:, qt, :], mvall[:, qt, 0:1],
                                rstdall[:, qt:qt + 1],
                                mybir.AluOpType.subtract,
                                mybir.AluOpType.mult)
        # FFN (transposed): hT = relu(W1p.T @ z.T + b1p); y = hT.T@Wr2 + r
        tp1 = pvp.tile([128, 512], F32, tag="po", name=f"tt{qt}")
        tp1b = tp1.bitcast(BF16)
        nc.tensor.transpose(tp1b[:, 0:128], z, ident_b)
        zT = tpool.tile([128, D], BF16, tag="zT", name=f"zT{qt}")
        nc.scalar.copy(zT, tp1b[:, 0:128])
        nc.tensor.matmul(tp1[:, 128:256], W1p, zT, start=True, stop=True)
        h1T = tpool.tile([128, D], BF16, tag="h1T", name=f"h1T{qt}")
        nc.scalar.activation(h1T, tp1[:, 128:256],
                             mybir.ActivationFunctionType.Relu,
                             bias=cols["b1p"])
        nc.tensor.matmul(tp1[:, 256:384], h1T, Wr2, start=True, stop=True)
        r = tpool.tile([128, D], BF16, tag="r", name=f"r{qt}")
        nc.vector.tensor_tensor(r, z, vecs["g0"], mybir.AluOpType.mult)
        nc.vector.tensor_tensor(r, r, vecs["bb"], mybir.AluOpType.add)
        nc.vector.scalar_tensor_tensor(Yfull[:, qt, :], tp1[:, 256:384],
                                       1.0, r, mybir.AluOpType.mult,
                                       mybir.AluOpType.add)
        st2 = small.tile([128, 6], F32, tag="bnst", name=f"st2_{qt}")
        nc.vector.bn_stats(st2, Yfull[:, qt, :])
        nc.vector.bn_aggr(mv2all[:, qt, :], st2)
    nc.scalar.activation(rstd2all, mv2all[:, :, 1],
                         mybir.ActivationFunctionType.Ln, bias=eps_t)
    nc.scalar.activation(rstd2all, rstd2all,
                         mybir.ActivationFunctionType.Exp, scale=-0.5)
    for qt in range(QT):
        z1 = tpool.tile([128, D], BF16, tag="z1", name=f"z1_{qt}")
        nc.vector.tensor_scalar(z1, Yfull[:, qt, :], mv2all[:, qt, 0:1],
                                rstd2all[:, qt:qt + 1],
                                mybir.AluOpType.subtract,
                                mybir.AluOpType.mult)
        o = tpool.tile([128, D], BF16, tag="o", name=f"o{qt}")
        nc.vector.tensor_tensor(o, z1, vecs["g1"], mybir.AluOpType.mult)
        nc.vector.tensor_tensor(o, o, vecs["be1"], mybir.AluOpType.add)
        nc.sync.dma_start(out_d[ts(qt, 128), :], o)

    ctx.close()


_NC_CACHE = {}


def _get_nc():
    if "nc" not in _NC_CACHE:
        _NC_CACHE["nc"] = _build_bass()
    return _NC_CACHE["nc"]


def _prep_inputs(Q, K, adj_mask, Wq, bq, Wk, bk, Wv, bv, Wr1, br1, Wr2, br2,
                 g0, be0, g1, be1):
    bf = ml_dtypes.bfloat16
    f32 = np.float32
    Q = np.asarray(Q, f32)
    K = np.asarray(K, f32)
    adj = np.asarray(adj_mask)
    g0f = np.asarray(g0, f32)
    be0f = np.asarray(be0, f32)
    Wr1f = np.asarray(Wr1, f32)
    shared = {
        "Wq": np.ascontiguousarray(Wq, f32),
        "Wk": np.ascontiguousarray(Wk, f32),
        "Wv": np.ascontiguousarray(Wv, f32),
        # g0 folded into Wr1; be0@Wr1+br1 folded into hidden bias
        "W1p": np.ascontiguousarray(g0f[:, None] * Wr1f).astype(bf),
        "Wr2b": np.ascontiguousarray(Wr2).astype(bf),
        "bk": np.ascontiguousarray(bk, f32).reshape(D, 1),
        "bq_s": (np.asarray(bq, f32) * SCALE).reshape(D, 1).copy(),
        "b1p": (be0f @ Wr1f + np.asarray(br1, f32)).reshape(D, 1).copy(),
        "bqv": (np.asarray(bq, f32) + np.asarray(bv, f32)).reshape(1, D),
        "g0": np.ascontiguousarray(g0, f32).reshape(1, D).astype(bf),
        "bb": (be0f + np.asarray(br2, f32)).reshape(1, D).astype(bf),
        "g1": np.ascontiguousarray(g1, f32).reshape(1, D).astype(bf),
        "be1": np.ascontiguousarray(be1, f32).reshape(1, D).astype(bf),
    }
    # mask layout per half: [h, mc, p, qt*128+j] = bias(adj[h, n0+qt*128+j,
    # mc*128+p]), stored bf16 {0,-1000} for the DMA-add masking
    mhalf = []
    for half in range(2):
        a = adj[:, half * NLOC:(half + 1) * NLOC, :]
        a = a.reshape(H, QT, 128, MC, 128)          # [h, qt, j, mc, p]
        a = np.ascontiguousarray(a.transpose(0, 3, 4, 1, 2))  # [h,mc,p,qt,j]
        a = a.reshape(H, MC, 128, NLOC)
        mhalf.append(np.where(a > 0, 0.0, -1000.0).astype(bf))
    in_maps = []
    for c in range(N_CORES):
        b, half = c // 2, c % 2
        im = dict(shared)
        im["KT"] = np.ascontiguousarray(K[b].T)
        im["QTr"] = np.ascontiguousarray(Q[b, half * NLOC:(half + 1) * NLOC].T)
        im["maskT"] = mhalf[half]
        in_maps.append(im)
    return in_maps


def _ensure_ntff_hook():
    """The agent image's antenv lacks axon_hooks, so the boot-time NTFF hook
    install silently degrades. Fabricate the module and install the hook via
    the boot module's own ctypes factory so trace=True works."""
    import sys
    import types
    try:
        from antenv.axon_hooks import get_axon_ntff_profile_hook  # noqa: F401
        return  # real module exists
    except ImportError:
        pass
    if "antenv.axon_hooks" in sys.modules:
        return
    from trn_agent_boot.trn_boot import _ntff_profile_via_ctypes
    hook = _ntff_profile_via_ctypes("/opt/axon/libaxon_pjrt.so")
    mod = types.ModuleType("antenv.axon_hooks")
    mod._hook = hook
    mod.get_axon_ntff_profile_hook = lambda: mod._hook
    mod.set_axon_ntff_profile_hook = lambda h: setattr(mod, "_hook", h)
    sys.modules["antenv.axon_hooks"] = mod


def run(trace=False, **inputs):
    nc = _get_nc()
    in_maps = _prep_inputs(**inputs)
    if trace:
        try:
            _ensure_ntff_hook()
        except Exception as e:
            print(f"ntff hook install failed ({e}); running without trace")
            trace = False
    res = run_bass_kernel_spmd(nc, in_maps, core_ids=list(range(N_CORES)),
                               trace=trace)
    out = np.empty((B, N, D), np.float32)
    for c in range(N_CORES):
        b, half = c // 2, c % 2
        out[b, half * NLOC:(half + 1) * NLOC] = \
            np.asarray(res.results[c]["out"], dtype=np.float32)
    return out, res


def kernel(**inputs) -> np.ndarray:
    out, _ = run(trace=False, **inputs)
    return out
